# revision 1
# baseline (speedup 1.0000x reference)
"""AttnBlock (GroupNorm + single-head spatial attention + residual) on 8
Trainium2 NeuronCores.

Sharding: data-parallel over B (4 batches) x 2-way query-sequence parallel =
8 shards. Each core gets the full x[b] (rolled so its query half is the
first 2048 spatial positions), computes GroupNorm + Q/K/V projections +
attention for its 2048 queries + output projection + residual, and writes a
[512, 2048] slice of the output.

Compute layout (per core, C=512, S=4096, Sq=2048):
  x        [c, s]   4 chunks of [128, 4096] f16 (channels on partitions;
                    loaded f16 -- h is f16-precision anyway -- the f32
                    residual half streams separately at low priority)
  h = GN(x)         4 chunks of [128, 4096] f16
  q = Wq h + bq     [128, 2048] f16 x4 (out-channels on partitions)
  k = Wk h + bk     [128, 4096] f16 x4
  vT = h^T Wv^T+bv  32 tiles of [128, 512] f16 (spatial on partitions!),
                    pre-scaled by 2^-6 so unnormalized attention fits f16
  scoresT[s,q] = k^T q   computed per (128-key-tile x 512-query-block) in
                 PSUM, exp()'d on ScalarE into SBUF f16 -- no transposes
                 anywhere: both AV operands already have s on partitions.
                 The key loop is software-pipelined (scores/exp of tile
                 st+1 issue before the AV matmuls of tile st) so the PE
                 never waits on the ScalarE exp.
  out'[c,q] += vT^T e    accumulated over all 32 key tiles in 4 PSUM banks
  Z[q]     += ones^T e   (full 128-wide ones lhsT: keeps fast-weight-load
                 mode AND broadcasts Z across partitions for free)
  out = x[:, :2048] + (Wo out')/Z + bo  -- normalization commutes with Wo,
                 so 1/Z runs on DVE off the PE critical path, and each
                 block's out-projection is deferred behind the next block's
                 key loop to hide its PSUM->f16 evacuation.

GroupNorm: per-quarter [sum, sumsq] pipelined with the x DMAs (sumsq on DVE
via scalar_tensor_tensor+accum, sum on ScalarE via Identity+accum), group
reduce/broadcast across the 16 channels of a group via tiny indicator
matmuls (pre-scaled host-side), rstd via exp(-0.5*ln(var+eps)) (stays in
the one preloaded ACT table set) plus a Newton step.

DMA discipline: the engines service all enqueued transfers round-robin
concurrently, so x chunks are stacked FIFO across the HW queues
(chunk-major) to finish in order; everything else (weights behind x on HW
queues, small constants and the dependency-gated residual on SW queues)
stays out of the critical HBM window.

Precision/speed tiering: the scores matmuls stay fp16 (q.k precision
feeds exp); the projections (q/k/vT/out) and the AV+Z matmuls run in
fp8e4m3 with perf_mode=DoubleRow -- two 128-rows of contraction packed
per PE pass, halving their instruction count. A DoubleRow matmul costs
the same ~216ns as one fp16 matmul (the 256-column LDWEIGHTS dominates),
so the win is purely instruction count; it also only materializes when
the pair dimension has a SMALL stride in both operands (h is stored
block-interleaved [p, pair, s-block, j, col] for this; applying DR to
the scores path with large pair-strides measured neutral and was
reverted). exp() is pre-shifted by 2^-4 so e never overflows fp8; the
shift cancels in the final (Wo att)/Z normalization. fp8 noise on the
softmax/value side averages across 4096 keys; end-to-end error is
~6.5e-3 of absmax (numpy-mirror-predicted before implementing).
fp16-everywhere variant: ~8e-5 at ~395us (kernel_fp16_395us.py.bak).
Measured: ~293us on hardware, PE busy ~240us of that.
"""
import numpy as np

import bass_rust
import concourse.bass as bass
import concourse.tile as tile
from concourse import mybir
from concourse.bass_utils import run_bass_kernel_spmd

F32 = mybir.dt.float32
F32R = mybir.dt.float32r
F16 = mybir.dt.float16
F8 = mybir.dt.float8e4
AF = mybir.ActivationFunctionType
ALU = mybir.AluOpType

B, C, H, W = 4, 512, 64, 64
S = H * W            # 4096 spatial positions (keys)
SQ = S // 2          # 2048 queries per core
CC = C // 128        # 4 channel chunks
ST = S // 128        # 32 key tiles
QB = SQ // 512       # 4 query blocks
NG = 32              # groups
GS = C // NG         # 16 channels per group
EPS = 1e-6
SCALE = 1.0 / float(np.sqrt(C))
VSCALE = 2.0 ** -6   # pre-scale on v and the Z-ones so |att| stays in f16
E8SHIFT = -4.0 * float(np.log(2.0))  # exp() pre-shift: e*2^-4 fits fp8e4m3


def _split_excess_waits(nc, max_waits=1):
    """walrus in this toolchain rejects instructions with >1 sync-wait.
    Hoist excess waits onto same-engine NOPs placed just before the
    instruction (engine streams are in-order, so this is equivalent)."""
    for f in nc.m.functions:
        for bb in f.blocks:
            out = []
            for inst in bb.instructions:
                si = inst.sync_info
                if si is not None and len(si.on_wait) > max_waits:
                    waits = list(si.on_wait)
                    plain = [w for w in waits if w.wait_reg is None]
                    special = [w for w in waits if w.wait_reg is not None]
                    n_keep = max(0, max_waits - len(special))
                    hoist = plain[: len(plain) - n_keep] if n_keep < len(plain) else []
                    keep = plain[len(hoist):] + special
                    if len(keep) > max_waits:
                        out.append(inst)
                        continue
                    for j, w in enumerate(hoist):
                        nop = mybir.InstNoOp(name=f"{inst.name}-wsplit{j}")
                        nop.engine = inst.engine
                        nop.sync_info = bass_rust.SyncInfo(on_wait=[w], on_update=[])
                        out.append(nop)
                    inst.sync_info = bass_rust.SyncInfo(
                        on_wait=keep, on_update=list(si.on_update))
                out.append(inst)
            bb.instructions = out


def _build(with_bv=True):
    nc = bass.Bass(trn_type="TRN2")

    x_d = nc.dram_tensor("x16", [C, S], F16, kind="ExternalInput")
    xr_d = nc.dram_tensor("xr32", [C, SQ], F32, kind="ExternalInput")
    w_d = {n: nc.dram_tensor(n, [C, C], F16, kind="ExternalInput")
           for n in ("wqT", "wkT", "wvT", "woT")}
    w8_d = {n: nc.dram_tensor(n, [128, 2, 2, C], mybir.dt.float8e4,
                              kind="ExternalInput")
            for n in ("w8q", "w8k", "w8v", "w8o")}
    bq_d = nc.dram_tensor("bqc", [128, CC], F32, kind="ExternalInput")
    bk_d = nc.dram_tensor("bkc", [128, CC], F32, kind="ExternalInput")
    bo_d = nc.dram_tensor("boc", [128, CC], F32, kind="ExternalInput")
    bv_d = nc.dram_tensor("bv16", [1, C], F16, kind="ExternalInput")
    ga_d = nc.dram_tensor("gammac", [128, CC], F32, kind="ExternalInput")
    be_d = nc.dram_tensor("betac", [128, CC], F32, kind="ExternalInput")
    ind_d = nc.dram_tensor("ind", [128, CC, NG], F32, kind="ExternalInput")
    indT_d = nc.dram_tensor("indT", [NG, CC, 128], F32, kind="ExternalInput")
    out_d = nc.dram_tensor("out", [CC, 128, SQ], F32, kind="ExternalOutput")

    with tile.TileContext(nc) as tc:
        from contextlib import ExitStack
        with ExitStack() as stack:
            const = stack.enter_context(tc.tile_pool(name="const", bufs=1))
            work = stack.enter_context(tc.tile_pool(name="work", bufs=3))
            p_res = stack.enter_context(tc.tile_pool(name="p_res", bufs=1))
            p_h = stack.enter_context(tc.tile_pool(name="p_h", bufs=1))

            # ---- constants (weight DMAs are emitted after the x DMAs so
            # they queue behind x on the DGE queues, not ahead of it) ----
            w_sb = {}
            for n in ("wqT", "wkT", "wvT", "woT"):
                t = const.tile([128, CC, C], F16, name=f"{n}_sb")
                w_sb[n] = t
            w8_sb = {}
            for n in ("w8q", "w8k", "w8v", "w8o"):
                t8 = const.tile([128, 2, 2, C], F8, name=f"{n}_sb")
                w8_sb[n] = t8

            def emit_weight_dmas():
                # behind x on the HW queues: x keeps full HBM bandwidth and
                # the weights still land well before the projections need them
                nc.sync.dma_start(out=w_sb["woT"][:],
                                  in_=w_d["woT"].rearrange(
                                      "(c p) o -> p c o", p=128))
                for n in ("w8q", "w8k", "w8v", "w8o"):
                    nc.sync.dma_start(out=w8_sb[n][:], in_=w8_d[n][:, :, :, :])

            bq_sb = const.tile([128, CC], F32, name="bq_sb")
            nc.gpsimd.dma_start(out=bq_sb[:], in_=bq_d[:, :])
            bk_sb = const.tile([128, CC], F32, name="bk_sb")
            nc.gpsimd.dma_start(out=bk_sb[:], in_=bk_d[:, :])
            bo_sb = const.tile([128, CC], F32, name="bo_sb")
            nc.gpsimd.dma_start(out=bo_sb[:], in_=bo_d[:, :])
            bv_sb = const.tile([1, C], F16, name="bv_sb")
            nc.gpsimd.dma_start(out=bv_sb[:], in_=bv_d[:, :])
            ga_sb = const.tile([128, CC], F32, name="ga_sb")
            nc.gpsimd.dma_start(out=ga_sb[:], in_=ga_d[:, :])
            be_sb = const.tile([128, CC], F32, name="be_sb")
            nc.gpsimd.dma_start(out=be_sb[:], in_=be_d[:, :])
            ind_sb = const.tile([128, CC, NG], F32, name="ind_sb")
            nc.gpsimd.dma_start(out=ind_sb[:], in_=ind_d[:, :, :])
            indT_sb = const.tile([NG, CC, 128], F32, name="indT_sb")
            nc.gpsimd.dma_start(out=indT_sb[:], in_=indT_d[:, :, :])

            ones_r16 = const.tile([1, 128], F16, name="ones_r16")
            nc.vector.memset(ones_r16[:], 1.0)
            # full-width ones pair-tile for the DoubleRow Z matmul: its
            # PSUM output is Z broadcast across all 128 partitions for free
            ones8 = const.tile([128, 2, 128], F8, name="ones8")
            nc.vector.memset(ones8[:], 1.0)
            e8b_sb = const.tile([128, 1], F32, name="e8b_sb")
            nc.vector.memset(e8b_sb[:], E8SHIFT)
            eps_sb = const.tile([NG, 1], F32, name="eps_sb")
            nc.vector.memset(eps_sb[:], EPS)

            h8 = p_h.tile([128, 2, S // 512, 2, 512], F8, name="h8")
            xres = p_res.tile([128, CC, SQ], F32, name="xres")

            # warm the ScalarE natural_log_exp table set while the input DMAs
            # are still in flight (the set load is ~2.7us and all ACT
            # functions used below -- Ln/Exp/Identity/Copy -- live in it)
            warm = work.tile([1, 2], F32, name="warm", tag="warm")
            nc.vector.memset(warm[:], 0.0)
            nc.scalar.activation(warm[:, 1:2], warm[:, 0:1], AF.Exp)

            # =========== Phase 1: load x + GroupNorm ===========
            with tc.tile_pool(name="p_x", bufs=1) as p_x, \
                 tc.tile_pool(name="ps_gn", bufs=2, space="PSUM") as ps_gn:
                xc = p_x.tile([128, CC, S], F16, name="xc")
                # x is loaded in f16: the GN stats and h are f16-precision
                # anyway, and halving the critical-path bytes halves the time
                # to first compute. The f32 residual half streams separately
                # on the SW queues (it isn't needed until the out-projection).
                # The DMA engines service all enqueued transfers round-robin
                # CONCURRENTLY, so chunks are stacked FIFO across the HW
                # queues (chunk-major) to finish in order, early.
                for i in range(CC):
                    for qq in range(4):
                        cols = slice(qq * 1024, (qq + 1) * 1024)
                        nc.sync.dma_start(out=xc[:, i, cols],
                                          in_=x_d[i * 128:(i + 1) * 128, cols])
                emit_weight_dmas()

                # per-channel [sum, sumsq], computed PER QUARTER so the stats
                # pipeline with the x DMAs instead of waiting for full
                # chunks. sumsq on DVE ((x*1)*x via scalar_tensor_tensor +
                # accum_out), sum on ScalarE (Identity + accum_out) -- the
                # two run in parallel and neither needs a new ACT table set.
                stats2 = []
                for i in range(CC):
                    s2q = work.tile([128, 2, 4], F32, name="s2q",
                                    tag="gn_s2q", bufs=4)
                    for qq in range(4):
                        qcols = slice(qq * 1024, (qq + 1) * 1024)
                        sq = p_x.tile([128, 1024], F16, name="sq", tag="sq",
                                      bufs=2)
                        nc.vector.scalar_tensor_tensor(
                            out=sq[:], in0=xc[:, i, qcols], scalar=1.0,
                            in1=xc[:, i, qcols], op0=ALU.mult, op1=ALU.mult,
                            accum_out=s2q[:, 1, qq:qq + 1])
                        sq2 = p_x.tile([128, 1024], F16, name="sq2",
                                       tag="sq2", bufs=2)
                        nc.scalar.activation(sq2[:], xc[:, i, qcols],
                                             AF.Identity,
                                             accum_out=s2q[:, 0, qq:qq + 1])
                    stats2.append(s2q)

                # reduce over the 16 channels of each group: indicator matmul
                # ([32, 2, 4] per-quarter partials), then fold the quarters
                psg = ps_gn.tile([NG, 2, 4], F32, name="psg")
                for i in range(CC):
                    nc.tensor.matmul(psg[:], ind_sb[:, i, :], stats2[i][:],
                                     start=(i == 0), stop=(i == CC - 1))
                # ind is pre-scaled by 1/(GS*S) host-side, so psg already
                # holds per-quarter [mean, E[x^2]] contributions
                gstat = work.tile([NG, 2], F32, name="gstat")  # [mean, E[x^2]]
                nc.vector.tensor_reduce(out=gstat[:], in_=psg[:],
                                        axis=mybir.AxisListType.X, op=ALU.add)

                # rstd_g = (var+eps)^-0.5 via exp(-0.5*ln(var+eps)) -- Ln and
                # Exp share the already-loaded table set (Sqrt would force a
                # set switch) -- plus one Newton step for full fp32 accuracy
                nve = work.tile([NG, 1], F32, name="nve")  # mean^2 - E[x^2]
                nc.vector.scalar_tensor_tensor(
                    out=nve[:], in0=gstat[:, 0:1], scalar=gstat[:, 0:1],
                    in1=gstat[:, 1:2], op0=ALU.mult, op1=ALU.subtract)
                lnv = work.tile([NG, 1], F32, name="lnv")
                nc.scalar.activation(lnv[:], nve[:], AF.Ln, scale=-1.0,
                                     bias=eps_sb[:])
                r0 = work.tile([NG, 1], F32, name="r0")
                nc.scalar.activation(r0[:], lnv[:], AF.Exp, scale=-0.5)
                ve = work.tile([NG, 1], F32, name="ve")
                nc.scalar.activation(ve[:], nve[:], AF.Identity, scale=-1.0,
                                     bias=eps_sb[:])
                r0sq = work.tile([NG, 1], F32, name="r0sq")
                nc.vector.tensor_mul(r0sq[:], r0[:], r0[:])
                t2 = work.tile([NG, 1], F32, name="t2")
                nc.vector.tensor_mul(t2[:], ve[:], r0sq[:])
                t3 = work.tile([NG, 1], F32, name="t3")
                nc.vector.tensor_scalar(out=t3[:], in0=t2[:], scalar1=-0.5,
                                        scalar2=1.5, op0=ALU.mult, op1=ALU.add)
                gv = work.tile([NG, 2], F32, name="gv")  # [mean_g, rstd_g]
                nc.vector.tensor_copy(gv[:, 0:1], gstat[:, 0:1])
                nc.vector.tensor_mul(gv[:, 1:2], r0[:], t3[:])

                # broadcast group stats back to channels; sc = rstd*gamma
                # and bi' = mean*sc - beta read the broadcast PSUM directly
                # (h = x*sc - bi' on DVE chunks; ACT chunks negate the bias)
                sc_bi = []
                for i in range(CC):
                    psb = ps_gn.tile([128, 2], F32, name="psb")
                    nc.tensor.matmul(psb[:], indT_sb[:, i, :], gv[:],
                                     start=True, stop=True)
                    sc_c = work.tile([128, 1], F32, name="sc_c", tag="gn_sc", bufs=4)
                    nc.vector.tensor_mul(sc_c[:], psb[:, 1:2], ga_sb[:, i:i + 1])
                    bi_c = work.tile([128, 1], F32, name="bi_c", tag="gn_bi", bufs=4)
                    nc.vector.scalar_tensor_tensor(
                        out=bi_c[:], in0=psb[:, 0:1], scalar=sc_c[:],
                        in1=be_sb[:, i:i + 1], op0=ALU.mult, op1=ALU.subtract)
                    if i % 2 == 0:
                        bn_c = work.tile([128, 1], F32, name="bn_c",
                                         tag="gn_bn", bufs=2)
                        nc.vector.tensor_scalar_mul(bn_c[:], bi_c[:], -1.0)
                        sc_bi.append((sc_c, bn_c))
                    else:
                        sc_bi.append((sc_c, bi_c))

                # h = x*scale + bias, cast to f16 -- split into halves and
                # alternate ScalarE/VectorE; all first halves go before the
                # second halves so the projections (which consume 512-col
                # blocks in order) can start as early as possible
                for hh in range(2):
                    cols = slice(hh * SQ, (hh + 1) * SQ)
                    for i in range(CC):
                        sc_c, bi_c = sc_bi[i]
                        hslc = h8[:, i // 2, 4 * hh:4 * hh + 4, i % 2, :]
                        if i % 2 == 0:
                            nc.scalar.activation(hslc,
                                                 xc[:, i, cols], AF.Identity,
                                                 bias=bi_c[:], scale=sc_c[:])
                        else:
                            nc.vector.tensor_scalar(
                                out=hslc, in0=xc[:, i, cols],
                                scalar1=sc_c[:], scalar2=bi_c[:],
                                op0=ALU.mult, op1=ALU.subtract)

            # =========== Phase 2: projections ===========
            p_kv = stack.enter_context(tc.tile_pool(name="p_kv", bufs=1))
            k16 = p_kv.tile([128, CC, S], F16, name="k16")
            q16 = p_kv.tile([128, CC, SQ], F16, name="q16")
            vT8 = p_kv.tile([128, ST, C], F8, name="vT8")

            with tc.tile_pool(name="ps_proj", bufs=3, space="PSUM") as ps_p:
                # q = WqT^T h (+bq): only the first SQ columns of h
                DRp = mybir.MatmulPerfMode.DoubleRow
                for oc in range(CC):
                    for qb in range(SQ // 512):
                        pt = ps_p.tile([128, 512], F32, name="pt", tag="pp")
                        cols = slice(qb * 512, (qb + 1) * 512)
                        for u in range(2):
                            nc.tensor.matmul(
                                pt[:],
                                w8_sb["w8q"][:, u, :, oc * 128:(oc + 1) * 128],
                                h8[:, u, qb, :, :],
                                start=(u == 0), stop=(u == 1), perf_mode=DRp)
                        nc.scalar.activation(q16[:, oc, cols], pt[:],
                                             AF.Identity, bias=bq_sb[:, oc:oc + 1])
                # k = WkT^T h (+bk): all S columns
                k_anchor = None
                for oc in range(CC):
                    for sb in range(S // 512):
                        pt = ps_p.tile([128, 512], F32, name="pt", tag="pp")
                        cols = slice(sb * 512, (sb + 1) * 512)
                        for u in range(2):
                            nc.tensor.matmul(
                                pt[:],
                                w8_sb["w8k"][:, u, :, oc * 128:(oc + 1) * 128],
                                h8[:, u, sb, :, :],
                                start=(u == 0), stop=(u == 1), perf_mode=DRp)
                        k_anchor = nc.scalar.activation(
                            k16[:, oc, cols], pt[:],
                            AF.Identity, bias=bk_sb[:, oc:oc + 1])
                # vT[s, c] = h[:, s]^T WvT (+bv broadcast via ones-matmul).
                # vT is stored pre-scaled by 2^-6 (and the Z-ones column uses
                # the same scale) so the unnormalized attention accumulator
                # stays comfortably inside f16 range; the scale cancels in
                # the final (Wo att)/Z normalization.
                for st in range(ST):
                    pt = ps_p.tile([128, 512], F32, name="pt", tag="pp")
                    scols = slice(st * 128, (st + 1) * 128)
                    ccol = slice((st % 4) * 128, (st % 4) * 128 + 128)
                    for u in range(2):
                        nc.tensor.matmul(pt[:], h8[:, u, st // 4, :, ccol],
                                         w8_sb["w8v"][:, u, :, :],
                                         start=(u == 0),
                                         stop=(u == 1 and not with_bv),
                                         perf_mode=DRp)
                    if with_bv:
                        nc.tensor.matmul(pt[:], ones_r16[:], bv_sb[:],
                                         start=False, stop=True)
                    nc.scalar.copy(vT8[:, st, :], pt[:])

            # residual stream: explicitly gated behind the k-projection so
            # it never competes with the x16/weight loads for HBM during the
            # startup window (it is first consumed by the out-projection)
            from concourse.bass import _add_dep_helper
            for i in range(CC):
                d = nc.gpsimd.dma_start(out=xres[:, i, :],
                                        in_=xr_d[i * 128:(i + 1) * 128, :])
                _add_dep_helper(d.ins, k_anchor.ins, True,
                                "xres stream deferred past startup")

            # =========== Phase 3: attention + out-projection ===========
            # att (= 2^-6 * sum_s e[s,q] v[:,s], unnormalized) is evacuated
            # to f16 right after the key loop; normalization by 1/Z happens
            # AFTER the out-projection (it commutes with Wo), so the
            # reciprocal/broadcast chain runs on DVE off the PE critical
            # path. The out-projection for block qb is emitted after block
            # qb+1's key loop so its PSUM->f16 dependency is fully hidden.
            with tc.tile_pool(name="ps_po", bufs=4, space="PSUM") as ps_po, \
                 tc.tile_pool(name="ps_z", bufs=1, space="PSUM") as ps_z, \
                 tc.tile_pool(name="ps_s", bufs=3, space="PSUM") as ps_s:

                def emit_outproj(qb, att8, rzb):
                    qcols = slice(qb * 512, (qb + 1) * 512)
                    for oc in range(CC):
                        pp = ps_s.tile([128, 512], F32, name="pp", tag="msum")
                        for u in range(2):
                            nc.tensor.matmul(
                                pp[:],
                                w8_sb["w8o"][:, u, :, oc * 128:(oc + 1) * 128],
                                att8[u][:],
                                start=(u == 0), stop=(u == 1),
                                perf_mode=mybir.MatmulPerfMode.DoubleRow)
                        t32 = work.tile([128, 512], F32, name="t32", tag="t32", bufs=2)
                        nc.vector.tensor_mul(t32[:], pp[:], rzb[:])
                        o32 = work.tile([128, 512], F32, name="o32", tag="o32", bufs=2)
                        nc.vector.scalar_tensor_tensor(
                            out=o32[:], in0=t32[:], scalar=bo_sb[:, oc:oc + 1],
                            in1=xres[:, oc, qcols], op0=ALU.add, op1=ALU.add)
                        nc.sync.dma_start(out=out_d[oc, :, qcols], in_=o32[:])

                NP = ST // 2   # key-tile pairs (fp8 DoubleRow packs 2)

                def emit_scores_pair(qb, t):
                    qcols = slice(qb * 512, (qb + 1) * 512)
                    e8p = work.tile([128, 2, 512], F8, name="e8p",
                                    tag="e8p", bufs=3)
                    for j in range(2):
                        st = 2 * t + j
                        pscore = ps_s.tile([128, 512], F32, name="pscore",
                                           tag="msum")
                        scols = slice(st * 128, (st + 1) * 128)
                        for ic in range(CC):
                            nc.tensor.matmul(pscore[:], k16[:, ic, scols],
                                             q16[:, ic, qcols],
                                             start=(ic == 0),
                                             stop=(ic == CC - 1))
                        # e' = exp(score/sqrt(C)) * 2^-4 so fp8e4m3 never
                        # overflows; the shift cancels against Z in the
                        # final normalization
                        nc.scalar.activation(e8p[:, j, :], pscore[:], AF.Exp,
                                             scale=SCALE, bias=e8b_sb[:])
                    return e8p

                def emit_av(po, pz, t, e8p):
                    DR = mybir.MatmulPerfMode.DoubleRow
                    for cc2 in range(CC):
                        nc.tensor.matmul(
                            po[cc2][:],
                            vT8[:, 2 * t:2 * t + 2, cc2 * 128:(cc2 + 1) * 128],
                            e8p[:],
                            start=(t == 0), stop=(t == NP - 1), perf_mode=DR)
                    nc.tensor.matmul(pz[:], ones8[:], e8p[:],
                                     start=(t == 0), stop=(t == NP - 1),
                                     perf_mode=DR)

                prev = None
                for qb in range(QB):
                    po = [ps_po.tile([128, 512], F32, name="po", tag="po")
                          for _ in range(CC)]
                    pz = ps_z.tile([128, 512], F32, name="pz", tag="pz")
                    # software-pipelined: scores/exp for pair t+1 are
                    # issued before the AV matmuls of pair t, so the PE
                    # never waits on the ScalarE exp.
                    e_prev = emit_scores_pair(qb, 0)
                    for t in range(1, NP):
                        e_cur = emit_scores_pair(qb, t)
                        emit_av(po, pz, t - 1, e_prev)
                        e_prev = e_cur
                        if t == NP // 2 and prev is not None:
                            # previous block's out-projection interleaves
                            # into the middle of this key loop: the PE absorbs
                            # its 8 matmuls where it is already the bottleneck
                            # and its DVE multiplies run while DVE is idle --
                            # instead of serializing on the kernel tail
                            emit_outproj(*prev)
                            prev = None
                    emit_av(po, pz, NP - 1, e_prev)
                    # att and Z are both scaled by 2^-4 here so the fp8
                    # att8 cast can never overflow (|po| can reach ~2300 vs
                    # fp8e4m3's 448 max); the scale cancels in (Wo att)/Z
                    zb = work.tile([128, 512], F32, name="zb", tag="zb", bufs=2)
                    nc.vector.tensor_scalar_mul(zb[:], pz[:], 2.0 ** -4)
                    rzb = work.tile([128, 512], F32, name="rzb", tag="rzb",
                                    bufs=2)
                    # evacuate att to fp8 pairs on ScalarE (frees po fast);
                    # the previous block's out-projection runs before this
                    # block's reciprocal so the PE's PSUM slots recycle
                    # without waiting on the 3us reciprocal
                    att8 = [work.tile([128, 2, 512], F8, name="att8",
                                      tag="att8", bufs=4) for _ in range(2)]
                    for cc2 in range(CC):
                        nc.scalar.mul(att8[cc2 // 2][:, cc2 % 2, :],
                                      po[cc2][:], 2.0 ** -4)
                    nc.vector.reciprocal(rzb[:], zb[:])
                    prev = (qb, att8, rzb)
                emit_outproj(*prev)

    _split_excess_waits(nc)
    return nc


_cache = {}


def _get_program(with_bv):
    key = ("nc", with_bv)
    if key not in _cache:
        _cache[key] = _build(with_bv)
    return _cache[key]


def kernel(x, gamma, beta, wq, bq, wk, bk, wv, bv, wo, bo, trace=False):
    x = np.asarray(x, dtype=np.float32)
    gamma = np.asarray(gamma, dtype=np.float32)
    beta = np.asarray(beta, dtype=np.float32)
    wq, wk, wv, wo = (np.asarray(a, dtype=np.float32) for a in (wq, wk, wv, wo))
    bq, bk, bv, bo = (np.asarray(a, dtype=np.float32) for a in (bq, bk, bv, bo))

    nc = _get_program(with_bv=bool(np.any(bv)))

    f8np = mybir.dt.np(F8)

    def pack8(w):
        wt = np.ascontiguousarray(w.T.astype(np.float32))
        return np.ascontiguousarray(
            wt.reshape(2, 2, 128, C).transpose(2, 0, 1, 3)).astype(f8np)

    shared = {
        "w8q": pack8(wq), "w8k": pack8(wk), "w8v": pack8(wv), "w8o": pack8(wo),
        "wqT": np.ascontiguousarray(wq.T).astype(np.float16),
        "wkT": np.ascontiguousarray(wk.T).astype(np.float16),
        "wvT": np.ascontiguousarray(wv.T).astype(np.float16),
        "woT": np.ascontiguousarray(wo.T).astype(np.float16),
        "bqc": np.ascontiguousarray(bq.reshape(CC, 128).T),
        "bkc": np.ascontiguousarray(bk.reshape(CC, 128).T),
        "boc": np.ascontiguousarray(bo.reshape(CC, 128).T),
        "bv16": bv.reshape(1, C).astype(np.float16),
        "gammac": np.ascontiguousarray(gamma.reshape(CC, 128).T),
        "betac": np.ascontiguousarray(beta.reshape(CC, 128).T),
    }
    ind = np.zeros((128, CC, NG), np.float32)
    indT = np.zeros((NG, CC, 128), np.float32)
    for i in range(CC):
        for p in range(128):
            g = (i * 128 + p) // GS
            ind[p, i, g] = 1.0 / (GS * S)
            indT[g, i, p] = 1.0
    shared["ind"] = ind
    shared["indT"] = indT

    in_maps = []
    for core in range(8):
        b, half = core // 2, core % 2
        xs = x[b].reshape(C, S)
        if half:
            xin = np.concatenate([xs[:, SQ:], xs[:, :SQ]], axis=1)
        else:
            xin = np.ascontiguousarray(xs)
        in_maps.append({"x16": xin.astype(np.float16),
                        "xr32": np.ascontiguousarray(xin[:, :SQ]), **shared})

    res = run_bass_kernel_spmd(nc, in_maps, core_ids=list(range(8)),
                               trace=trace)
    _cache["last_exec_time_ns"] = res.exec_time_ns

    y = np.empty((B, C, S), np.float32)
    for core in range(8):
        b, half = core // 2, core % 2
        y[b, :, half * SQ:(half + 1) * SQ] = \
            res.results[core]["out"].reshape(C, SQ)
    return y.reshape(B, C, H, W)

